# revision 4
# baseline (speedup 1.0000x reference)
"""Bresenham (border-ring) attention kernel for Trainium2, 8 NeuronCores.

Computation (per full input):
    att  = einsum('bchw,c->bhw', x, w) + b        # 1x1 conv to 1 channel
    att  = sigmoid(att)
    mask = border ring of the HxW rectangle       # 1 on border, 0 inside
    out  = x * (att * (1 + mask))[:, None]

Strategy:
  - Pure data parallel: batch 16 -> 2 per core across 8 cores.
  - Per core, per batch: x[b] is [C=256, HW=50176] (f32, HW contiguous).
    Tile spatially in superblocks of FD columns, channels split in two
    halves of 128 partitions.
  - att (contraction over C) via PE matmuls: lhsT = [w_half, w_half]
    ([128, 2]) so PSUM rows 0 and 1 both accumulate att.  A third K=1
    matmul adds the mask offset vector M (0 on border, -60 interior) to
    row 1 only.  ACT computes s = sigmoid(att + bias) for both rows in
    one instruction (bias = conv_b per-partition vector).
  - combined = sigmoid(att+b)*(1+mask) == s_row0 + s_row1 exactly
    (border: 2*sigmoid; interior: sigmoid + sigmoid(att-60) ~ sigmoid).
    The sum AND the broadcast across 128 partitions happen in one K=2
    ones-matmul: bc = ones[2,128]^T @ s[2,N].
  - out tile = x tile * bc via DVE tensor_tensor (PSUM operand).

All engines stay well under the DMA roofline (~205 MB/core at ~360 GB/s
=> ~570 us); kernel is HBM-bandwidth-bound.
"""

import numpy as np

import concourse.bacc as bacc
import concourse.bass as bass
import concourse.tile as tile
from concourse import mybir
from concourse.bass_utils import run_bass_kernel_spmd

B, C, H, W = 16, 256, 224, 224
HW = H * W  # 50176
NCORES = 8
BLOC = B // NCORES  # 2

FD = 3584            # superblock free dim (spatial columns per tile)
SUB = 512            # matmul subtile (one PSUM bank of f32)
NSUB = FD // SUB     # 7
NBLK = HW // FD      # 14
NEG = -60.0          # interior mask offset (sigmoid(x-60) == 0 in f32 sums)

F32 = mybir.dt.float32

# stash of the last BassKernelResults (test.py reads exec_time_ns from here)
LAST_RESULTS = None
_NC_CACHE = {}


def _build_nc():
    nc = bacc.Bacc("TRN2", debug=False)

    x = nc.dram_tensor("x", [BLOC, C, HW], F32, kind="ExternalInput")
    w01 = nc.dram_tensor("w01", [128, 2], F32, kind="ExternalInput")
    w11 = nc.dram_tensor("w11", [128, 2], F32, kind="ExternalInput")
    sel = nc.dram_tensor("sel", [1, 2], F32, kind="ExternalInput")
    ones2 = nc.dram_tensor("ones2", [2, 128], F32, kind="ExternalInput")
    bias2 = nc.dram_tensor("bias2", [2, 1], F32, kind="ExternalInput")
    mv = nc.dram_tensor("mv", [NBLK, 1, FD], F32, kind="ExternalInput")
    out = nc.dram_tensor("out", [BLOC, C, HW], F32, kind="ExternalOutput")

    # view [BLOC, C, HW] as [BLOC, p=128, h=2, n]: c = h*128 + p
    x_r = x.ap().rearrange("b (h p) n -> b p h n", h=2)
    out_r = out.ap().rearrange("b (h p) n -> b p h n", h=2)

    with tile.TileContext(nc) as tc:
        with (
            tc.tile_pool(name="consts", bufs=1) as consts,
            tc.tile_pool(name="xin", bufs=2) as xin_pool,
            tc.tile_pool(name="oout", bufs=2) as out_pool,
            tc.tile_pool(name="spool", bufs=2) as s_pool,
            tc.tile_pool(name="mvp", bufs=2) as mv_pool,
            tc.tile_pool(name="psA", bufs=2, space="PSUM") as psA,
            tc.tile_pool(name="psB", bufs=4, space="PSUM") as psB,
        ):
            w01_t = consts.tile([128, 2], F32)
            nc.sync.dma_start(out=w01_t[:], in_=w01.ap())
            w11_t = consts.tile([128, 2], F32)
            nc.sync.dma_start(out=w11_t[:], in_=w11.ap())
            sel_t = consts.tile([1, 2], F32)
            nc.sync.dma_start(out=sel_t[:], in_=sel.ap())
            ones2_t = consts.tile([2, 128], F32)
            nc.sync.dma_start(out=ones2_t[:], in_=ones2.ap())
            bias2_t = consts.tile([2, 1], F32)
            nc.sync.dma_start(out=bias2_t[:], in_=bias2.ap())

            for b in range(BLOC):
                for blk in range(NBLK):
                    n0 = blk * FD
                    xt = xin_pool.tile([128, 2, FD], F32)
                    nc.sync.dma_start(out=xt[:], in_=x_r[b, :, :, n0:n0 + FD])
                    mv_t = mv_pool.tile([1, FD], F32)
                    nc.sync.dma_start(out=mv_t[:], in_=mv.ap()[blk])
                    ot = out_pool.tile([128, 2, FD], F32)
                    st = s_pool.tile([2, FD], F32)

                    for j in range(NSUB):
                        js = slice(j * SUB, (j + 1) * SUB)
                        ps_att = psA.tile([2, SUB], F32)
                        nc.tensor.matmul(
                            ps_att[:], w01_t[:], xt[:, 0, js],
                            start=True, stop=False,
                        )
                        nc.tensor.matmul(
                            ps_att[:], w11_t[:], xt[:, 1, js],
                            start=False, stop=False,
                        )
                        nc.tensor.matmul(
                            ps_att[:], sel_t[:], mv_t[:, js],
                            start=False, stop=True,
                        )
                        nc.scalar.activation(
                            out=st[:, js],
                            in_=ps_att[:],
                            func=mybir.ActivationFunctionType.Sigmoid,
                            bias=bias2_t[:],
                            scale=1.0,
                        )
                        ps_bc = psB.tile([128, SUB], F32)
                        nc.tensor.matmul(
                            ps_bc[:], ones2_t[:], st[:, js],
                            start=True, stop=True,
                        )
                        nc.vector.tensor_mul(ot[:, 0, js], xt[:, 0, js], ps_bc[:])
                        nc.vector.tensor_mul(ot[:, 1, js], xt[:, 1, js], ps_bc[:])

                    nc.sync.dma_start(out=out_r[b, :, :, n0:n0 + FD], in_=ot[:])

    nc.compile()
    return nc


def _host_consts(conv_w, conv_b):
    w = np.asarray(conv_w, dtype=np.float32).reshape(C)
    w01 = np.repeat(w[:128, None], 2, axis=1).copy()       # [128, 2]
    w11 = np.repeat(w[128:, None], 2, axis=1).copy()       # [128, 2]
    sel = np.array([[0.0, 1.0]], dtype=np.float32)         # [1, 2]
    ones2 = np.ones((2, 128), dtype=np.float32)            # [2, 128]
    bias2 = np.full((2, 1), np.asarray(conv_b).reshape(-1)[0], dtype=np.float32)

    ys = np.arange(H)[:, None]
    xs = np.arange(W)[None, :]
    border = (ys == 0) | (ys == H - 1) | (xs == 0) | (xs == W - 1)
    mvec = np.where(border, 0.0, NEG).astype(np.float32).reshape(HW)
    mv = mvec.reshape(NBLK, 1, FD).copy()
    return dict(w01=w01, w11=w11, sel=sel, ones2=ones2, bias2=bias2, mv=mv)


def kernel(x, conv_w, conv_b):
    global LAST_RESULTS
    x = np.ascontiguousarray(np.asarray(x, dtype=np.float32))
    assert x.shape == (B, C, H, W), x.shape

    if "nc" not in _NC_CACHE:
        _NC_CACHE["nc"] = _build_nc()
    nc = _NC_CACHE["nc"]

    consts = _host_consts(conv_w, conv_b)
    x_flat = x.reshape(B, C, HW)

    in_maps = []
    for i in range(NCORES):
        m = {"x": np.ascontiguousarray(x_flat[i * BLOC:(i + 1) * BLOC])}
        m.update(consts)
        in_maps.append(m)

    res = run_bass_kernel_spmd(nc, in_maps, list(range(NCORES)))
    LAST_RESULTS = res

    out = np.concatenate(
        [r["out"].reshape(BLOC, C, H, W) for r in res.results], axis=0
    )
    return out
